# revision 6
# baseline (speedup 1.0000x reference)
"""Multi-head attention (B=2, S=4096, D=512, H=8) on 8 Trainium2 NeuronCores.

Sharding: core c handles batch b = c // 4 and head-group g = c % 4 (2 heads =
columns/rows [128g : 128g+128] of the projection weights).  Host pre-transposes
X (no on-device DMA-transposes), pre-casts to fp16, and folds bv/bo into the
unshard (softmax-average makes the V bias additive: out += bv @ Wo + bo, exact).

Device pipeline per core (one flattened software pipeline over all
(query-block, key-tile) bodies, ~1.03us/chunk, ACT- and PE-co-paced):
  A) xT chunks [128 din, <=1024 s] stream in (plain DMAs, k/v prioritized,
     projection emission deadline-scheduled against DMA arrival);
     kT/qT = W.T @ xT + bias ([128 dout, S] fp16, dout = 2 heads x 64); V is
     projected straight to natural [keys, dout] tiles (lhsT = xT chunk,
     rhs = Wv) and scattered into vaug0/vaug1 (ones column at col 64 / col 0
     for softmax denominators).
  B) chunk = one key tile = one head PAIR: QK emitted as adjacent K=64
     matmul pairs (head0 rows 0:64 / head1 rows 64:128, auto tile_position
     row tiling) -> both heads stream through the PE concurrently (2x);
     logits -> PSUM [128,1024] (3-deep lg ring; depth 3 absorbs the
     interleaved phase-C/proj/transpose allocations without collapsing the
     ring to single-buffering).  exp(0.125 * logits) runs on ACT for most
     chunks and as a one-pass DVE Schraudolph fp16 bit-trick (int16 out
     bitcast to fp16, max rel err ~3%) for 4 of 32 chunks per qb to offload
     the saturated ACT; PV accumulates [uctx.T | denom] per head in two
     dedicated PSUM banks (never recycled: recycling made the denominator
     reciprocals gate the next qb's PV through the in-order PE queue).
  C) denominator rows are PE-transposed into a lg-ring tile, reciprocated on
     DVE into rd; out[st] = (uctx_h0.T @ Wo[0:64]) * rd0 + (uctx_h1.T @
     Wo[64:128]) * rd1 (another row-tiled pair), scaled on DVE, fp16 out on
     the Activation HWDGE queue (so outputs never queue behind inputs).

PSUM: lg ring 3 x 2 banks + pv0 + pv1 = 8 banks exactly.  The flush and
phase-C lg-ring allocations are placed immediately AFTER the Schraudolph
chunks (ci 5/13/21/29 -> insertions at 5, 6, 14, 22, 30): an insertion
shifts the ring phase so later QK allocations wait on a chunk's consumer,
and aligning that consumer to be the DVE (which finishes while ACT is busy)
instead of an ACT exp removed ~1.05us ACT bubbles per insertion (-12us).

Measured: ~314.4us (baseline 388.9us); rel err ~1.2e-2 (gate 2e-2; the
pure-ACT exp variant is ~8us slower at 7.7e-4 — set SCHRAUD = set()).
"""

import os

import numpy as np

import concourse.bass as bass
import concourse.tile as tile
from concourse import bacc, mybir
from concourse.bass_utils import run_bass_kernel_spmd
from concourse.masks import make_identity

P = 128
D = 512
GD = 128  # head-group width: 2 heads x 64
HD = 64
S = 4096
B_FULL = 2
N_CORES = 8
NT = S // P  # 32 key tiles
QB = S // 512  # 8 query blocks
F32 = mybir.dt.float32
F16 = mybir.dt.float16
EXP = mybir.ActivationFunctionType.Exp
MUL = mybir.AluOpType.mult
ADD = mybir.AluOpType.add


def _emit(tc, io):
    nc = tc.nc
    xq, xk, xv, wq, wk, wv, wo, bq, bk, out = io

    with (
        tc.tile_pool(name="persist", bufs=1) as pp,
        tc.tile_pool(name="lgp", bufs=3, space="PSUM") as lgp,
        tc.tile_pool(name="pvp", bufs=1, space="PSUM") as pvp,
        tc.tile_pool(name="xp", bufs=2) as xp,
        tc.tile_pool(name="ptp", bufs=4) as ptp,
        tc.tile_pool(name="ucp", bufs=3) as ucp,
        tc.tile_pool(name="tmp", bufs=2) as tmpp,
        tc.tile_pool(name="obp", bufs=3) as obp,
    ):
        ident32 = pp.tile([P, P], F32, name="ident32")
        make_identity(nc, ident32)

        # weights + biases
        wqs = pp.tile([P, 4, GD], F16, name="wqs")
        wks = pp.tile([P, 4, GD], F16, name="wks")
        wvs = pp.tile([P, 4, GD], F16, name="wvs")
        nc.sync.dma_start(wqs, wq.rearrange("(t p) m -> p t m", p=P))
        nc.sync.dma_start(wks, wk.rearrange("(t p) m -> p t m", p=P))
        nc.sync.dma_start(wvs, wv.rearrange("(t p) m -> p t m", p=P))
        wos = pp.tile([P, D], F16, name="wos")
        nc.sync.dma_start(wos, wo)
        bqs = pp.tile([P, 1], F32, name="bqs")
        bks = pp.tile([P, 1], F32, name="bks")
        nc.sync.dma_start(bqs, bq[:, None])
        nc.sync.dma_start(bks, bk[:, None])

        # persistent activations
        kT = pp.tile([P, S], F16, name="kT")
        qT = pp.tile([P, S], F16, name="qT")
        vaug0 = pp.tile([P, NT, P], F16, name="vaug0")
        vaug1 = pp.tile([P, NT, P], F16, name="vaug1")
        nc.gpsimd.memset(vaug0[:, :, HD:P], 0.0)
        nc.gpsimd.memset(vaug0[:, :, HD : HD + 1], 1.0)
        nc.gpsimd.memset(vaug1[:, :, 0:HD], 0.0)
        nc.gpsimd.memset(vaug1[:, :, 0:1], 1.0)
        uctx16 = pp.tile([P, S], F16, name="uctx16")
        rd = pp.tile([P, 2, NT], F32, name="rd")

        # ---- input DMAs: 5 column-chunks per tensor (512,1024x3,512) so the
        # first s-block arrives fast; k/v prioritized, q trailing ------------
        CHUNK_COLS = [(0, 512), (512, 1536), (1536, 2560), (2560, 3584), (3584, 4096)]
        SB_CHUNK = {0: (0, 0), 1: (1, 0), 2: (1, 512), 3: (2, 0),
                    4: (2, 512), 5: (3, 0), 6: (3, 512), 7: (4, 0)}
        xin = {}

        def dma_chunk(which, dt, c):
            src = {"q": xq, "k": xk, "v": xv}[which]
            lo, hi = CHUNK_COLS[c]
            t = xp.tile([P, hi - lo], F16, tag=f"x{which}{dt}", name="xin",
            bufs=5 if which == "q" else 2)
            nc.sync.dma_start(t, src[dt * P : (dt + 1) * P, lo:hi])
            xin[(which, dt, c)] = t

        dma_order = [("k", 0), ("q", 0), ("v", 0), ("k", 1), ("v", 1),
                     ("k", 2), ("v", 2), ("q", 1), ("k", 3), ("v", 3),
                     ("k", 4), ("v", 4), ("q", 2), ("q", 3), ("q", 4)]
        for which, c in dma_order:
            for dt in range(4):
                dma_chunk(which, dt, c)

        # ---- projections ---------------------------------------------------
        def emit_kq_proj(which, sb):
            w, dest, bias = {
                "q": (wqs, qT, bqs),
                "k": (wks, kT, bks),
            }[which]
            c, off = SB_CHUNK[sb]
            acc = lgp.tile([P, 512], F32, tag="lg", name="acc")
            for dt in range(4):
                nc.tensor.matmul(
                    acc,
                    lhsT=w[:, dt, :],
                    rhs=xin[(which, dt, c)][:, off : off + 512],
                    start=(dt == 0),
                    stop=(dt == 3),
                )
            nc.vector.tensor_scalar_add(
                dest[:, sb * 512 : (sb + 1) * 512], acc[:], bias[:]
            )

        def emit_v_proj(sb, half):
            c, off = SB_CHUNK[sb]
            vacc = lgp.tile([P, 512], F32, tag="lg", name="vacc")
            for j in (2 * half, 2 * half + 1):
                kt = 4 * sb + j
                lo = off + j * P
                for dt in range(4):
                    nc.tensor.matmul(
                        vacc[:, (j % 2) * P : (j % 2 + 1) * P],
                        lhsT=xin[("v", dt, c)][:, lo : lo + P],
                        rhs=wvs[:, dt, :],
                        start=(dt == 0),
                        stop=(dt == 3),
                    )
                nc.vector.tensor_copy(
                    out=vaug0[:, kt, 0:HD],
                    in_=vacc[:, (j % 2) * P : (j % 2) * P + HD],
                )
                nc.vector.tensor_copy(
                    out=vaug1[:, kt, HD:P],
                    in_=vacc[:, (j % 2) * P + HD : (j % 2 + 1) * P],
                )

        # proj deadlines in global body indices (32 bodies per qb, 1 kt each)
        k_deadline = {b: max(0, 4 * b - 3) for b in range(1, 8)}
        v_deadline = {(0, 0): 0, (0, 1): 1}
        v_deadline.update({(b, hf): 4 * b + 2 * hf - 2
                           for b in range(1, 8) for hf in (0, 1)})
        q_deadline = {b: 16 if b == 1 else 32 * (b - 1) + 16 for b in range(1, 8)}

        # ---- attention: one chunk = one key tile = one head PAIR ----------
        chunks = [[(kt, 0), (kt, 1)] for kt in range(NT)]
        NCH = len(chunks)  # 32

        def emit_qk_chunk(qb, ci, lg):
            ch = chunks[ci]
            for i, (kt, h) in enumerate(ch):
                nc.tensor.matmul(
                    lg[:, i * 512 : (i + 1) * 512],
                    lhsT=kT[h * HD : (h + 1) * HD, kt * P : (kt + 1) * P],
                    rhs=qT[h * HD : (h + 1) * HD, qb * 512 : (qb + 1) * 512],
                    start=True,
                    stop=True,
                )

        pending = []  # (qb, uc0, uc1) awaiting denom transpose + phase C

        def emit_flush_and_phase_c():
            tqb, uc0, uc1 = pending.pop(0)
            # denominator transposes ride the lg ring (one [128,1024] tile
            # holds all 8) so the pv banks are never recycled — recycling
            # them made the recips gate the next qb's PV in the in-order PE
            # queue (1.8us ACT bubble per qb)
            tps = lgp.tile([P, 1024], F32, tag="lg", name="tps")
            for sl in range(4):
                nc.tensor.transpose(
                    tps[:, sl * P : (sl + 1) * P],
                    uc0[:, sl * P : (sl + 1) * P],
                    ident32,
                )
                nc.tensor.transpose(
                    tps[:, (4 + sl) * P : (5 + sl) * P],
                    uc1[:, sl * P : (sl + 1) * P],
                    ident32,
                )
            for sl in range(4):
                st = 4 * tqb + sl
                nc.vector.reciprocal(
                    rd[:, 0, st : st + 1], tps[:, sl * P + HD : sl * P + HD + 1]
                )
                nc.vector.reciprocal(
                    rd[:, 1, st : st + 1], tps[:, (4 + sl) * P : (4 + sl) * P + 1]
                )

        def emit_phase_c_st(st, act_scale=False):
            stcols = slice(st * P, (st + 1) * P)
            oo = lgp.tile([P, 1024], F32, tag="lg", name="oo")
            nc.tensor.matmul(
                oo[:, 0:512],
                lhsT=uctx16[0:HD, stcols],
                rhs=wos[0:HD, :],
                start=True,
                stop=True,
            )
            nc.tensor.matmul(
                oo[:, 512:1024],
                lhsT=uctx16[HD:P, stcols],
                rhs=wos[HD:P, :],
                start=True,
                stop=True,
            )
            t0 = tmpp.tile([P, 512], F32, tag="t0", name="t0")
            if act_scale:
                # tail only: ACT is idle there, DVE is the tail chain
                nc.scalar.activation(
                    t0, oo[:, 0:512],
                    mybir.ActivationFunctionType.Copy,
                    scale=rd[:, 0, st : st + 1],
                )
            else:
                nc.vector.tensor_scalar_mul(
                    t0, oo[:, 0:512], rd[:, 0, st : st + 1]
                )
            ob = obp.tile([P, 512], F16, tag="ob", name="ob")
            nc.vector.scalar_tensor_tensor(
                out=ob,
                in0=oo[:, 512:1024],
                scalar=rd[:, 1, st : st + 1],
                in1=t0[:],
                op0=MUL,
                op1=ADD,
            )
            nc.scalar.dma_start(out[st * P : (st + 1) * P, :], ob)

        emit_kq_proj("k", 0)
        emit_kq_proj("q", 0)

        # ---- flattened software pipeline over all (qb, chunk) bodies ------
        bodies = [(qb, ci) for qb in range(QB) for ci in range(NCH)]
        lg_tiles = {}

        def emit_qk(j):
            qb, ci = bodies[j]
            lg = lgp.tile([P, 1024], F32, tag="lg", name="lg")
            emit_qk_chunk(qb, ci, lg)
            lg_tiles[j] = lg

        # chunks whose exp runs as a one-pass DVE Schraudolph (fp16 bit trick,
        # max rel err ~3%) to offload the saturated ACT engine
        SCHRAUD = {5, 13, 21, 29}
        SCH_A = 184.6649652337873   # 0.125 * 1024 / ln(2)
        SCH_B = 15315.5             # 15360 - 44.5 (minimax bias)

        pv = {}
        uc0 = None
        pv_backlog = []
        emit_qk(0)
        for j, (qb, ci) in enumerate(bodies):
            ch = chunks[ci]
            lg = lg_tiles.pop(j)
            if ci in SCHRAUD:
                si = ptp.tile([P, 1024], mybir.dt.int16, tag="pt", name="si")
                nc.vector.tensor_scalar(
                    si, lg[:], SCH_A, SCH_B, MUL, ADD
                )
                ptt = si.bitcast(F16)
            else:
                ptt = ptp.tile([P, 1024], F16, tag="pt", name="ptt")
                nc.scalar.activation(ptt[:], lg[:], EXP, scale=0.125)
            if j + 1 < len(bodies):
                emit_qk(j + 1)
            # interleaved work AFTER the next QK so ACT never starves
            for b in list(k_deadline):
                if k_deadline[b] <= j:
                    emit_kq_proj("k", b)
                    del k_deadline[b]
            for key in list(v_deadline):
                if v_deadline[key] <= j:
                    emit_v_proj(*key)
                    del v_deadline[key]
            for b in list(q_deadline):
                if q_deadline[b] <= j:
                    emit_kq_proj("q", b)
                    del q_deadline[b]
            if qb >= 1:
                if ci == 5:
                    emit_flush_and_phase_c()
                elif ci in (6, 14, 22, 30):
                    emit_phase_c_st(4 * (qb - 1) + (ci - 6) // 8)
            if ci == 0:
                pv[0] = pvp.tile([P, 512], F32, tag="pv0", name="pv0")
                pv[1] = pvp.tile([P, 512], F32, tag="pv1", name="pv1")
            qcols = slice(qb * 512, (qb + 1) * 512)
            if True:
                for bci, bch, bptt in [(ci, ch, ptt)]:
                    for i, (kt, h) in enumerate(bch):
                        nc.tensor.matmul(
                            pv[h],
                            lhsT=vaug0[:, kt, :] if h == 0 else vaug1[:, kt, :],
                            rhs=bptt[:, i * 512 : (i + 1) * 512],
                            start=(kt == 0),
                            stop=(kt == NT - 1),
                        )
                        if bci == NCH - 1:
                            # this head's accumulation just finished; drain it
                            uc = ucp.tile([P, 512], F32, tag=f"uc{h}", name="uc")
                            nc.vector.tensor_copy(out=uc, in_=pv[h][:])
                            if h == 0:
                                nc.vector.tensor_copy(
                                    out=uctx16[0:HD, qcols], in_=uc[0:HD, :]
                                )
                                uc0 = uc
                            else:
                                nc.vector.tensor_copy(
                                    out=uctx16[HD:P, qcols], in_=uc[HD:P, :]
                                )
                                pending.append((qb, uc0, uc))

        # tail: last iter's denominators + remaining phase C tiles
        emit_flush_and_phase_c()
        for st in range(4 * (QB - 1), 4 * QB):
            emit_phase_c_st(st, act_scale=True)


def build(enable_asserts=False):
    nc = bacc.Bacc(
        "TRN2",
        target_bir_lowering=False,
        debug=False,
        enable_asserts=enable_asserts,
        num_devices=N_CORES,
    )
    xq = nc.dram_tensor("xq", [D, S], F16, kind="ExternalInput").ap()
    xk = nc.dram_tensor("xk", [D, S], F16, kind="ExternalInput").ap()
    xv = nc.dram_tensor("xv", [D, S], F16, kind="ExternalInput").ap()
    wq = nc.dram_tensor("wq", [D, GD], F16, kind="ExternalInput").ap()
    wk = nc.dram_tensor("wk", [D, GD], F16, kind="ExternalInput").ap()
    wv = nc.dram_tensor("wv", [D, GD], F16, kind="ExternalInput").ap()
    wo = nc.dram_tensor("wo", [GD, D], F16, kind="ExternalInput").ap()
    bq = nc.dram_tensor("bq", [GD], F32, kind="ExternalInput").ap()
    bk = nc.dram_tensor("bk", [GD], F32, kind="ExternalInput").ap()
    out = nc.dram_tensor("out", [S, D], F16, kind="ExternalOutput").ap()
    io = (xq, xk, xv, wq, wk, wv, wo, bq, bk, out)
    with tile.TileContext(nc) as tc:
        _emit(tc, io)
    nc.compile()
    return nc


def make_in_maps(queries, keys, values, Wq, bq, Wk, bk, Wv, bv, Wo, bo):
    f16 = lambda a: np.ascontiguousarray(
        np.asarray(a, dtype=np.float32).astype(np.float16)
    )
    f16T = lambda a: np.ascontiguousarray(
        np.asarray(a, dtype=np.float32).astype(np.float16).T
    )
    f32 = lambda a: np.ascontiguousarray(np.asarray(a, dtype=np.float32))
    in_maps = []
    for c in range(N_CORES):
        b, g = divmod(c, 4)
        sl = slice(g * GD, (g + 1) * GD)
        in_maps.append(
            {
                "xq": f16T(queries[b]),
                "xk": f16T(keys[b]),
                "xv": f16T(values[b]),
                "wq": f16(np.asarray(Wq)[:, sl]),
                "wk": f16(np.asarray(Wk)[:, sl]),
                "wv": f16(np.asarray(Wv)[:, sl]),
                "wo": f16(np.asarray(Wo)[sl, :]),
                "bq": f32(np.asarray(bq)[sl]),
                "bk": f32(np.asarray(bk)[sl]),
            }
        )
    return in_maps


_NC = None
last_results = None


def kernel(queries, keys, values, Wq, bq, Wk, bk, Wv, bv, Wo, bo):
    global _NC, last_results
    if _NC is None:
        _NC = build()
    in_maps = make_in_maps(
        queries, keys, values, Wq, bq, Wk, bk, Wv, bv, Wo, bo
    )
    res = run_bass_kernel_spmd(
        _NC,
        in_maps,
        core_ids=list(range(N_CORES)),
        trace=bool(int(os.environ.get("MHA_TRACE", "0"))),
    )
    last_results = res
    outs = [
        np.asarray(res.results[c]["out"], dtype=np.float32)
        for c in range(N_CORES)
    ]
    full = np.empty((B_FULL, S, D), dtype=np.float32)
    bo32 = np.asarray(bo, dtype=np.float32)
    bv32 = np.asarray(bv, dtype=np.float32)
    wo32 = np.asarray(Wo, dtype=np.float32)
    extra = bo32 + bv32 @ wo32  # softmax-avg makes the V bias additive
    for b in range(B_FULL):
        full[b] = outs[4 * b] + outs[4 * b + 1] + outs[4 * b + 2] + outs[4 * b + 3]
        full[b] += extra
    return full


# revision 8
# speedup vs baseline: 1.0009x; 1.0009x over previous
"""Multi-head attention (B=2, S=4096, D=512, H=8) on 8 Trainium2 NeuronCores.

Sharding: core c handles batch b = c // 4 and head-group g = c % 4 (2 heads =
columns/rows [128g : 128g+128] of the projection weights).  Host pre-transposes
X (no on-device DMA-transposes), pre-casts to fp16, and folds bv/bo into the
unshard (softmax-average makes the V bias additive: out += bv @ Wo + bo, exact).

Device pipeline per core (one flattened software pipeline over all
(query-block, key-tile) bodies, ~1.03us/chunk, ACT- and PE-co-paced):
  A) xT chunks [128 din, <=1024 s] stream in (plain DMAs, k/v prioritized,
     projection emission deadline-scheduled against DMA arrival);
     kT/qT = W.T @ xT + bias ([128 dout, S] fp16, dout = 2 heads x 64); V is
     projected straight to natural [keys, dout] tiles (lhsT = xT chunk,
     rhs = Wv) and scattered into vaug0/vaug1 (ones column at col 64 / col 0
     for softmax denominators).
  B) chunk = one key tile = one head PAIR: QK emitted as adjacent K=64
     matmul pairs (head0 rows 0:64 / head1 rows 64:128, auto tile_position
     row tiling) -> both heads stream through the PE concurrently (2x);
     logits -> PSUM [128,1024] (3-deep lg ring; depth 3 absorbs the
     interleaved phase-C/proj/transpose allocations without collapsing the
     ring to single-buffering).  exp(0.125 * logits) runs on ACT for most
     chunks and as a one-pass DVE Schraudolph fp16 bit-trick (int16 out
     bitcast to fp16, max rel err ~3%) for 4 of 32 chunks per qb to offload
     the saturated ACT; PV accumulates [uctx.T | denom] per head in two
     dedicated PSUM banks (never recycled: recycling made the denominator
     reciprocals gate the next qb's PV through the in-order PE queue).
  C) denominator rows are PE-transposed into a lg-ring tile, reciprocated on
     DVE into rd; out[st] = (uctx_h0.T @ Wo[0:64]) * rd0 + (uctx_h1.T @
     Wo[64:128]) * rd1 (another row-tiled pair), scaled on DVE, fp16 out on
     the Activation HWDGE queue (so outputs never queue behind inputs).

PSUM: lg ring 3 x 2 banks + pv0 + pv1 = 8 banks exactly.  The flush and
phase-C lg-ring allocations are placed immediately AFTER the Schraudolph
chunks (ci 5/13/21/29 -> insertions at 5, 6, 14, 22, 30): an insertion
shifts the ring phase so later QK allocations wait on a chunk's consumer,
and aligning that consumer to be the DVE (which finishes while ACT is busy)
instead of an ACT exp removed ~1.05us ACT bubbles per insertion (-12us).

Measured: ~314.4us (baseline 388.9us); rel err ~1.2e-2 (gate 2e-2; the
pure-ACT exp variant is ~8us slower at 7.7e-4 — set SCHRAUD = set()).
"""

import os

import numpy as np

import concourse.bass as bass
import concourse.tile as tile
from concourse import bacc, mybir
from concourse.bass_utils import run_bass_kernel_spmd
from concourse.masks import make_identity

P = 128
D = 512
GD = 128  # head-group width: 2 heads x 64
HD = 64
S = 4096
B_FULL = 2
N_CORES = 8
NT = S // P  # 32 key tiles
QB = S // 512  # 8 query blocks
F32 = mybir.dt.float32
F16 = mybir.dt.float16
EXP = mybir.ActivationFunctionType.Exp
MUL = mybir.AluOpType.mult
ADD = mybir.AluOpType.add


def _emit(tc, io):
    nc = tc.nc
    xq, xk, xv, wq, wk, wv, wo, bq, bk, out = io

    with (
        tc.tile_pool(name="persist", bufs=1) as pp,
        tc.tile_pool(name="lgp", bufs=3, space="PSUM") as lgp,
        tc.tile_pool(name="pvp", bufs=1, space="PSUM") as pvp,
        tc.tile_pool(name="xp", bufs=2) as xp,
        tc.tile_pool(name="ptp", bufs=4) as ptp,
        tc.tile_pool(name="ucp", bufs=3) as ucp,
        tc.tile_pool(name="tmp", bufs=2) as tmpp,
        tc.tile_pool(name="obp", bufs=3) as obp,
    ):
        ident32 = pp.tile([P, P], F32, name="ident32")
        make_identity(nc, ident32)

        # weights + biases
        wqs = pp.tile([P, 4, GD], F16, name="wqs")
        wks = pp.tile([P, 4, GD], F16, name="wks")
        wvs = pp.tile([P, 4, GD], F16, name="wvs")
        nc.sync.dma_start(wqs, wq.rearrange("(t p) m -> p t m", p=P))
        nc.sync.dma_start(wks, wk.rearrange("(t p) m -> p t m", p=P))
        nc.sync.dma_start(wvs, wv.rearrange("(t p) m -> p t m", p=P))
        wos = pp.tile([P, D], F16, name="wos")
        nc.sync.dma_start(wos, wo)
        bqs = pp.tile([P, 1], F32, name="bqs")
        bks = pp.tile([P, 1], F32, name="bks")
        nc.sync.dma_start(bqs, bq[:, None])
        nc.sync.dma_start(bks, bk[:, None])

        # persistent activations
        kT = pp.tile([P, S], F16, name="kT")
        qT = pp.tile([P, S], F16, name="qT")
        vaug0 = pp.tile([P, NT, P], F16, name="vaug0")
        vaug1 = pp.tile([P, NT, P], F16, name="vaug1")
        nc.gpsimd.memset(vaug0[:, :, HD:P], 0.0)
        nc.gpsimd.memset(vaug0[:, :, HD : HD + 1], 1.0)
        nc.gpsimd.memset(vaug1[:, :, 0:HD], 0.0)
        nc.gpsimd.memset(vaug1[:, :, 0:1], 1.0)
        uctx16 = pp.tile([P, S], F16, name="uctx16")
        rd = pp.tile([P, 2, NT], F32, name="rd")

        # ---- input DMAs: 5 column-chunks per tensor (512,1024x3,512) so the
        # first s-block arrives fast; k/v prioritized, q trailing ------------
        CHUNK_COLS = [(0, 512), (512, 1536), (1536, 2560), (2560, 3584), (3584, 4096)]
        SB_CHUNK = {0: (0, 0), 1: (1, 0), 2: (1, 512), 3: (2, 0),
                    4: (2, 512), 5: (3, 0), 6: (3, 512), 7: (4, 0)}
        xin = {}

        def dma_chunk(which, dt, c):
            src = {"q": xq, "k": xk, "v": xv}[which]
            lo, hi = CHUNK_COLS[c]
            t = xp.tile([P, hi - lo], F16, tag=f"x{which}{dt}", name="xin",
            bufs=5 if which == "q" else 2)
            nc.sync.dma_start(t, src[dt * P : (dt + 1) * P, lo:hi])
            xin[(which, dt, c)] = t

        dma_order = [("k", 0), ("q", 0), ("v", 0), ("k", 1), ("v", 1),
                     ("k", 2), ("v", 2), ("q", 1), ("k", 3), ("v", 3),
                     ("k", 4), ("v", 4), ("q", 2), ("q", 3), ("q", 4)]
        for which, c in dma_order:
            for dt in range(4):
                dma_chunk(which, dt, c)

        # ---- projections ---------------------------------------------------
        def emit_kq_proj(which, sb):
            w, dest, bias = {
                "q": (wqs, qT, bqs),
                "k": (wks, kT, bks),
            }[which]
            c, off = SB_CHUNK[sb]
            acc = lgp.tile([P, 512], F32, tag="lg", name="acc")
            for dt in range(4):
                nc.tensor.matmul(
                    acc,
                    lhsT=w[:, dt, :],
                    rhs=xin[(which, dt, c)][:, off : off + 512],
                    start=(dt == 0),
                    stop=(dt == 3),
                )
            nc.vector.tensor_scalar_add(
                dest[:, sb * 512 : (sb + 1) * 512], acc[:], bias[:]
            )

        def emit_v_proj(sb, half):
            c, off = SB_CHUNK[sb]
            vacc = lgp.tile([P, 512], F32, tag="lg", name="vacc")
            for j in (2 * half, 2 * half + 1):
                kt = 4 * sb + j
                lo = off + j * P
                for dt in range(4):
                    nc.tensor.matmul(
                        vacc[:, (j % 2) * P : (j % 2 + 1) * P],
                        lhsT=xin[("v", dt, c)][:, lo : lo + P],
                        rhs=wvs[:, dt, :],
                        start=(dt == 0),
                        stop=(dt == 3),
                    )
                nc.vector.tensor_copy(
                    out=vaug0[:, kt, 0:HD],
                    in_=vacc[:, (j % 2) * P : (j % 2) * P + HD],
                )
                nc.vector.tensor_copy(
                    out=vaug1[:, kt, HD:P],
                    in_=vacc[:, (j % 2) * P + HD : (j % 2 + 1) * P],
                )

        # proj deadlines in global body indices (32 bodies per qb, 1 kt each)
        k_deadline = {b: max(0, 4 * b - 3) for b in range(1, 8)}
        v_deadline = {(0, 0): 0, (0, 1): 1}
        v_deadline.update({(b, hf): 4 * b + 2 * hf - 2
                           for b in range(1, 8) for hf in (0, 1)})
        q_deadline = {b: 16 if b == 1 else 32 * (b - 1) + 16 for b in range(1, 8)}

        # ---- attention: one chunk = one key tile = one head PAIR ----------
        chunks = [[(kt, 0), (kt, 1)] for kt in range(NT)]
        NCH = len(chunks)  # 32

        def emit_qk_chunk(qb, ci, lg):
            ch = chunks[ci]
            for i, (kt, h) in enumerate(ch):
                nc.tensor.matmul(
                    lg[:, i * 512 : (i + 1) * 512],
                    lhsT=kT[h * HD : (h + 1) * HD, kt * P : (kt + 1) * P],
                    rhs=qT[h * HD : (h + 1) * HD, qb * 512 : (qb + 1) * 512],
                    start=True,
                    stop=True,
                )

        pending = []  # (qb, uc0, uc1) awaiting denom transpose + phase C

        def emit_flush_and_phase_c():
            tqb, uc0, uc1 = pending.pop(0)
            # denominator transposes ride the lg ring (one [128,1024] tile
            # holds all 8) so the pv banks are never recycled — recycling
            # them made the recips gate the next qb's PV in the in-order PE
            # queue (1.8us ACT bubble per qb)
            tps = lgp.tile([P, 1024], F32, tag="lg", name="tps")
            for sl in range(4):
                nc.tensor.transpose(
                    tps[:, sl * P : (sl + 1) * P],
                    uc0[:, sl * P : (sl + 1) * P],
                    ident32,
                )
                nc.tensor.transpose(
                    tps[:, (4 + sl) * P : (5 + sl) * P],
                    uc1[:, sl * P : (sl + 1) * P],
                    ident32,
                )
            for sl in range(4):
                st = 4 * tqb + sl
                nc.vector.reciprocal(
                    rd[:, 0, st : st + 1], tps[:, sl * P + HD : sl * P + HD + 1]
                )
                nc.vector.reciprocal(
                    rd[:, 1, st : st + 1], tps[:, (4 + sl) * P : (4 + sl) * P + 1]
                )

        def emit_phase_c_st(st, act_scale=False):
            stcols = slice(st * P, (st + 1) * P)
            oo = lgp.tile([P, 1024], F32, tag="lg", name="oo")
            nc.tensor.matmul(
                oo[:, 0:512],
                lhsT=uctx16[0:HD, stcols],
                rhs=wos[0:HD, :],
                start=True,
                stop=True,
            )
            nc.tensor.matmul(
                oo[:, 512:1024],
                lhsT=uctx16[HD:P, stcols],
                rhs=wos[HD:P, :],
                start=True,
                stop=True,
            )
            t0 = tmpp.tile([P, 512], F32, tag="t0", name="t0")
            if act_scale:
                # tail only: ACT is idle there, DVE is the tail chain
                nc.scalar.activation(
                    t0, oo[:, 0:512],
                    mybir.ActivationFunctionType.Copy,
                    scale=rd[:, 0, st : st + 1],
                )
            else:
                nc.vector.tensor_scalar_mul(
                    t0, oo[:, 0:512], rd[:, 0, st : st + 1]
                )
            ob = obp.tile([P, 512], F16, tag="ob", name="ob")
            nc.vector.scalar_tensor_tensor(
                out=ob,
                in0=oo[:, 512:1024],
                scalar=rd[:, 1, st : st + 1],
                in1=t0[:],
                op0=MUL,
                op1=ADD,
            )
            nc.scalar.dma_start(out[st * P : (st + 1) * P, :], ob)

        emit_kq_proj("k", 0)
        emit_kq_proj("q", 0)

        # ---- flattened software pipeline over all (qb, chunk) bodies ------
        bodies = [(qb, ci) for qb in range(QB) for ci in range(NCH)]
        lg_tiles = {}

        def emit_qk(j):
            qb, ci = bodies[j]
            lg = lgp.tile([P, 1024], F32, tag="lg", name="lg")
            emit_qk_chunk(qb, ci, lg)
            lg_tiles[j] = lg

        # chunks whose exp runs as a one-pass DVE Schraudolph (fp16 bit trick,
        # max rel err ~3%) to offload the saturated ACT engine
        SCHRAUD = {5, 9, 13, 17, 21, 25, 29}
        SCH_A = 184.6649652337873   # 0.125 * 1024 / ln(2)
        SCH_B = 15315.5             # 15360 - 44.5 (minimax bias)

        pv = {}
        uc0 = None
        pv_backlog = []
        emit_qk(0)
        for j, (qb, ci) in enumerate(bodies):
            ch = chunks[ci]
            lg = lg_tiles.pop(j)
            if ci in SCHRAUD:
                si = ptp.tile([P, 1024], mybir.dt.int16, tag="pt", name="si")
                nc.vector.tensor_scalar(
                    si, lg[:], SCH_A, SCH_B, MUL, ADD
                )
                ptt = si.bitcast(F16)
            else:
                ptt = ptp.tile([P, 1024], F16, tag="pt", name="ptt")
                nc.scalar.activation(ptt[:], lg[:], EXP, scale=0.125)
            if j + 1 < len(bodies):
                emit_qk(j + 1)
            # interleaved work AFTER the next QK so ACT never starves
            for b in list(k_deadline):
                if k_deadline[b] <= j:
                    emit_kq_proj("k", b)
                    del k_deadline[b]
            for key in list(v_deadline):
                if v_deadline[key] <= j:
                    emit_v_proj(*key)
                    del v_deadline[key]
            for b in list(q_deadline):
                if q_deadline[b] <= j:
                    emit_kq_proj("q", b)
                    del q_deadline[b]
            if qb >= 1:
                if ci == 5:
                    emit_flush_and_phase_c()
                elif ci in (10, 18, 26, 30):
                    emit_phase_c_st(4 * (qb - 1) + {10: 0, 18: 1, 26: 2, 30: 3}[ci])
            if ci == 0:
                pv[0] = pvp.tile([P, 512], F32, tag="pv0", name="pv0")
                pv[1] = pvp.tile([P, 512], F32, tag="pv1", name="pv1")
            qcols = slice(qb * 512, (qb + 1) * 512)
            if True:
                for bci, bch, bptt in [(ci, ch, ptt)]:
                    for i, (kt, h) in enumerate(bch):
                        nc.tensor.matmul(
                            pv[h],
                            lhsT=vaug0[:, kt, :] if h == 0 else vaug1[:, kt, :],
                            rhs=bptt[:, i * 512 : (i + 1) * 512],
                            start=(kt == 0),
                            stop=(kt == NT - 1),
                        )
                        if bci == NCH - 1:
                            # this head's accumulation just finished; drain it
                            uc = ucp.tile([P, 512], F32, tag=f"uc{h}", name="uc")
                            nc.vector.tensor_copy(out=uc, in_=pv[h][:])
                            if h == 0:
                                nc.vector.tensor_copy(
                                    out=uctx16[0:HD, qcols], in_=uc[0:HD, :]
                                )
                                uc0 = uc
                            else:
                                nc.vector.tensor_copy(
                                    out=uctx16[HD:P, qcols], in_=uc[HD:P, :]
                                )
                                pending.append((qb, uc0, uc))

        # tail: last iter's denominators + remaining phase C tiles
        emit_flush_and_phase_c()
        for st in range(4 * (QB - 1), 4 * QB):
            emit_phase_c_st(st, act_scale=True)


def build(enable_asserts=False):
    nc = bacc.Bacc(
        "TRN2",
        target_bir_lowering=False,
        debug=False,
        enable_asserts=enable_asserts,
        num_devices=N_CORES,
    )
    xq = nc.dram_tensor("xq", [D, S], F16, kind="ExternalInput").ap()
    xk = nc.dram_tensor("xk", [D, S], F16, kind="ExternalInput").ap()
    xv = nc.dram_tensor("xv", [D, S], F16, kind="ExternalInput").ap()
    wq = nc.dram_tensor("wq", [D, GD], F16, kind="ExternalInput").ap()
    wk = nc.dram_tensor("wk", [D, GD], F16, kind="ExternalInput").ap()
    wv = nc.dram_tensor("wv", [D, GD], F16, kind="ExternalInput").ap()
    wo = nc.dram_tensor("wo", [GD, D], F16, kind="ExternalInput").ap()
    bq = nc.dram_tensor("bq", [GD], F32, kind="ExternalInput").ap()
    bk = nc.dram_tensor("bk", [GD], F32, kind="ExternalInput").ap()
    out = nc.dram_tensor("out", [S, D], F16, kind="ExternalOutput").ap()
    io = (xq, xk, xv, wq, wk, wv, wo, bq, bk, out)
    with tile.TileContext(nc) as tc:
        _emit(tc, io)
    nc.compile()
    return nc


def make_in_maps(queries, keys, values, Wq, bq, Wk, bk, Wv, bv, Wo, bo):
    f16 = lambda a: np.ascontiguousarray(
        np.asarray(a, dtype=np.float32).astype(np.float16)
    )
    f16T = lambda a: np.ascontiguousarray(
        np.asarray(a, dtype=np.float32).astype(np.float16).T
    )
    f32 = lambda a: np.ascontiguousarray(np.asarray(a, dtype=np.float32))
    in_maps = []
    for c in range(N_CORES):
        b, g = divmod(c, 4)
        sl = slice(g * GD, (g + 1) * GD)
        in_maps.append(
            {
                "xq": f16T(queries[b]),
                "xk": f16T(keys[b]),
                "xv": f16T(values[b]),
                "wq": f16(np.asarray(Wq)[:, sl]),
                "wk": f16(np.asarray(Wk)[:, sl]),
                "wv": f16(np.asarray(Wv)[:, sl]),
                "wo": f16(np.asarray(Wo)[sl, :]),
                "bq": f32(np.asarray(bq)[sl]),
                "bk": f32(np.asarray(bk)[sl]),
            }
        )
    return in_maps


_NC = None
last_results = None


def kernel(queries, keys, values, Wq, bq, Wk, bk, Wv, bv, Wo, bo):
    global _NC, last_results
    if _NC is None:
        _NC = build()
    in_maps = make_in_maps(
        queries, keys, values, Wq, bq, Wk, bk, Wv, bv, Wo, bo
    )
    res = run_bass_kernel_spmd(
        _NC,
        in_maps,
        core_ids=list(range(N_CORES)),
        trace=bool(int(os.environ.get("MHA_TRACE", "0"))),
    )
    last_results = res
    outs = [
        np.asarray(res.results[c]["out"], dtype=np.float32)
        for c in range(N_CORES)
    ]
    full = np.empty((B_FULL, S, D), dtype=np.float32)
    bo32 = np.asarray(bo, dtype=np.float32)
    bv32 = np.asarray(bv, dtype=np.float32)
    wo32 = np.asarray(Wo, dtype=np.float32)
    extra = bo32 + bv32 @ wo32  # softmax-avg makes the V bias additive
    for b in range(B_FULL):
        full[b] = outs[4 * b] + outs[4 * b + 1] + outs[4 * b + 2] + outs[4 * b + 3]
        full[b] += extra
    return full
